# revision 5
# baseline (speedup 1.0000x reference)
"""TRN2 kernel for HAKMEM entangled complex attention, fully on-device.

8 NeuronCores, head-parallel (2 heads/core). Per core:
  - QKV projections (bf16 matmuls, entanglement+perm folded into weights)
  - rope via host-precomputed coefficient tiles (C1/C2), swap via DMA
  - phase on K via per-partition scalars (doubled angle)
  - flash attention over (sigma, tau) tiles in transposed orientation:
      arT/aiT score matmuls (K-stacked 128-contraction), custom-DVE
      mag^2, ACT sqrt/exp (table-switch batched per sigma), causal mask,
      PV + ones-matmul row sums, normalize
  - per-sigma AllGather of normalized yT, column-sharded output projection
Host does: weight folding, rope tables, bias folding (bv -> bo), final
column concat + output bias.
"""
import sys
sys.path.insert(0, "/opt/trn_rl_repo")
import os
import numpy as np
import ml_dtypes

BF = ml_dtypes.bfloat16
DIM, H, Dh, ROT, S = 1024, 16, 64, 32, 2048
NCORES = 8
HPC = H // NCORES              # heads per core = 2
PERM = np.concatenate([np.arange(0, ROT, 2), np.arange(1, ROT, 2),
                       np.arange(ROT, Dh)])

_CACHE = {}


# --------------------------------------------------------------------------
# host-side preparation
# --------------------------------------------------------------------------

def _prep(inputs):
    E = np.asarray(inputs["entanglement_matrix"], np.float32)

    def fold(W, b):
        W4 = np.asarray(W, np.float32).reshape(H, Dh, DIM)[:, PERM, :]
        b4 = np.asarray(b, np.float32).reshape(H, Dh)[:, PERM]
        W4 = np.einsum("hx,hdD->xdD", E, W4)
        b4 = np.einsum("hx,hd->xd", E, b4)
        return W4, b4

    Wq_r, bq_r = fold(inputs["Wq_r"], inputs["bq_r"])
    Wq_i, bq_i = fold(inputs["Wq_i"], inputs["bq_i"])
    Wk_r, bk_r = fold(inputs["Wk_r"], inputs["bk_r"])
    Wk_i, bk_i = fold(inputs["Wk_i"], inputs["bk_i"])
    Wv_r = np.asarray(inputs["Wv_r"], np.float32).reshape(H, Dh, DIM)
    Wv_i = np.asarray(inputs["Wv_i"], np.float32).reshape(H, Dh, DIM)
    bv_r = np.asarray(inputs["bv_r"], np.float32).reshape(DIM)
    bv_i = np.asarray(inputs["bv_i"], np.float32).reshape(DIM)

    pos = np.arange(S, dtype=np.float64)
    freqs = np.asarray(inputs["rotary_freqs"], np.float32).astype(np.float64)
    ang = np.outer(pos, freqs)
    cosv = np.cos(ang).astype(np.float32).T          # [16, S]
    sinv = np.sin(ang).astype(np.float32).T
    C1 = np.concatenate([cosv, cosv], axis=0)        # [32, S]
    C2 = np.concatenate([-sinv, sinv], axis=0)
    C14 = np.ascontiguousarray(np.tile(C1, (4, 1))).astype(BF)   # [128, S]
    C24 = np.ascontiguousarray(np.tile(C2, (4, 1))).astype(BF)

    ph = np.asarray(inputs["phase_shifts"], np.float32).reshape(H, Dh)[:, PERM]
    c2 = np.cos(2 * ph.astype(np.float64)).astype(np.float32)
    s2 = np.sin(2 * ph.astype(np.float64)).astype(np.float32)

    eps = 1 / (1 + np.exp(-float(np.asarray(inputs["circle_epsilon"])))) * 0.03
    magsc = float((1 + eps * eps) / Dh)
    temp = max(np.exp(float(np.asarray(inputs["attention_temperature"]))), 0.1)
    alpha = float(1 / (1 + np.exp(
        -float(np.asarray(inputs["interference_strength"])))) / temp)

    Wo_r = np.asarray(inputs["Wo_r"], np.float32)
    Wo_i = np.asarray(inputs["Wo_i"], np.float32)
    bo_r = np.asarray(inputs["bo_r"], np.float32) + Wo_r @ bv_r
    bo_i = np.asarray(inputs["bo_i"], np.float32) + Wo_i @ bv_i

    x_r = np.asarray(inputs["real"], np.float32)[0]
    x_i = np.asarray(inputs["imag"], np.float32)[0]
    xTr = np.ascontiguousarray(x_r.T).astype(BF)     # [DIM, S]
    xTi = np.ascontiguousarray(x_i.T).astype(BF)
    xtr = np.ascontiguousarray(xTr.reshape(8, 128, S).transpose(1, 0, 2)
                               .reshape(128, 8 * S))
    xti = np.ascontiguousarray(xTi.reshape(8, 128, S).transpose(1, 0, 2)
                               .reshape(128, 8 * S))

    maskt = (np.arange(128)[None, :] >= np.arange(128)[:, None]).astype(BF)

    def slab(W4, h):          # [Dh, DIM] -> lhsT chunks [128, 8*64]
        w = W4[h].T.astype(BF)            # [DIM, 64]
        return np.ascontiguousarray(
            w.reshape(8, 128, 64).transpose(1, 0, 2).reshape(128, 8 * 64))

    in_maps = []
    for c in range(NCORES):
        h0, h1 = 2 * c, 2 * c + 1
        m = {"xtr": xtr, "xti": xti, "c14": C14, "c24": C24,
             "maskt": np.ascontiguousarray(maskt)}
        for hh, h in ((0, h0), (1, h1)):
            m[f"wqr{hh}"] = slab(Wq_r, h)
            m[f"wqi{hh}"] = slab(Wq_i, h)
            m[f"wkr{hh}"] = slab(Wk_r, h)
            m[f"wki{hh}"] = slab(Wk_i, h)
            m[f"biasq{hh}"] = np.ascontiguousarray(
                np.concatenate([bq_r[h], bq_i[h]]).reshape(128, 1))
            m[f"biask{hh}"] = np.ascontiguousarray(
                np.concatenate([bk_r[h], bk_i[h]]).reshape(128, 1))
            m[f"ph1{hh}"] = np.ascontiguousarray(
                np.concatenate([s2[h], c2[h]]).reshape(128, 1))
            m[f"ph2{hh}"] = np.ascontiguousarray(
                np.concatenate([c2[h], -s2[h]]).reshape(128, 1))
        wv_r = np.stack([Wv_r[h0].T, Wv_r[h1].T], axis=-1)  # [DIM, 64, 2]
        wv_i = np.stack([Wv_i[h0].T, Wv_i[h1].T], axis=-1)
        m["wvr"] = np.ascontiguousarray(
            wv_r.transpose(0, 2, 1).reshape(8, 128, 128)
            .transpose(1, 0, 2).reshape(128, 8 * 128)).astype(BF)
        m["wvi"] = np.ascontiguousarray(
            wv_i.transpose(0, 2, 1).reshape(8, 128, 128)
            .transpose(1, 0, 2).reshape(128, 8 * 128)).astype(BF)
        cols = slice(c * 128, (c + 1) * 128)
        wor = Wo_r[cols, :].T.astype(BF)          # [DIM(in), 128]
        woi = Wo_i[cols, :].T.astype(BF)
        m["wor"] = np.ascontiguousarray(
            wor.reshape(8, 128, 128).transpose(1, 0, 2).reshape(128, 8 * 128))
        m["woi"] = np.ascontiguousarray(
            woi.reshape(8, 128, 128).transpose(1, 0, 2).reshape(128, 8 * 128))
        in_maps.append(m)
    return in_maps, bo_r, bo_i, magsc, alpha


# --------------------------------------------------------------------------
# custom DVE op registration (idempotent)
# --------------------------------------------------------------------------

def _get_mag2():
    from concourse import dve_ops as DO
    from concourse.dve_spec import Spec, Src0, Src1, C0, C1, sq, lower as dlow
    from concourse.dve_uop import DveOpSpec
    for op in DO.OPS:
        if op.name == "MAG2_ANT":
            return op
    spec = Spec(body=(sq(Src0) + sq(Src1)) * C0 + C1,
                reference=lambda in0, in1, s0, s1, imm2: (
                    in0.astype(np.float32) ** 2
                    + in1.astype(np.float32) ** 2) * s0 + s1)
    row = DO._CUSTOM_DVE_ROW_BASE + len(DO.OPS)
    shas = {}
    for ver in ("v3", "v4"):
        s = DveOpSpec(name="MAG2_ANT", opcode=row,
                      uops=dlow(spec, ver=ver), rd1_en=True)
        shas[ver] = s.sha(ver)
    op = DO.DveOp("MAG2_ANT", spec, subdim=False, uops_sha=shas)
    DO.OPS.append(op)
    DO.CUSTOM_DVE_SPECS["MAG2_ANT"] = spec
    DO._SUB_OPCODE_FOR_NAME["MAG2_ANT"] = row
    return op


# --------------------------------------------------------------------------
# device kernel
# --------------------------------------------------------------------------

def _build_nc(magsc, alpha):
    import concourse.tile as tile
    from concourse import bacc, mybir
    F32, BF16 = mybir.dt.float32, mybir.dt.bfloat16
    AF = mybir.ActivationFunctionType
    MAG2 = _get_mag2()

    nc = bacc.Bacc("TRN2", target_bir_lowering=False, debug=False,
                   num_devices=NCORES)
    dt_in = {}

    def din(name, shape, dt=BF16):
        dt_in[name] = nc.dram_tensor(name, shape, dt, kind="ExternalInput").ap()
        return dt_in[name]

    xtr = din("xtr", [128, 8 * S]); xti = din("xti", [128, 8 * S])
    c14 = din("c14", [128, S]);     c24 = din("c24", [128, S])
    maskt = din("maskt", [128, 128])
    for hh in range(HPC):
        for w in ("wqr", "wqi", "wkr", "wki"):
            din(f"{w}{hh}", [128, 8 * 64])
        din(f"biasq{hh}", [128, 1], F32); din(f"biask{hh}", [128, 1], F32)
        din(f"ph1{hh}", [128, 1], F32);   din(f"ph2{hh}", [128, 1], F32)
    wvr = din("wvr", [128, 8 * 128]); wvi = din("wvi", [128, 8 * 128])
    wor = din("wor", [128, 8 * 128]); woi = din("woi", [128, 8 * 128])
    out_r = nc.dram_tensor("out_r", [S, 128], F32, kind="ExternalOutput").ap()
    out_i = nc.dram_tensor("out_i", [S, 128], F32, kind="ExternalOutput").ap()

    with tile.TileContext(nc) as tc:
        with tc.tile_pool(name="qk", bufs=1) as qk, \
             tc.tile_pool(name="ps_sc", bufs=4, space="PSUM") as ps_sc, \
             tc.tile_pool(name="ps_yt", bufs=2, space="PSUM") as ps_yt, \
             tc.tile_pool(name="ps_mx", bufs=2, space="PSUM") as ps_mx, \
             tc.tile_pool(name="dram", bufs=1, space="DRAM") as dram:

            # persistent tiles
            t_c14 = qk.tile([128, S], BF16, tag="c14")
            t_c24 = qk.tile([128, S], BF16, tag="c24")
            nc.sync.dma_start(t_c14[:], c14[:])
            nc.sync.dma_start(t_c24[:], c24[:])
            t_mask = qk.tile([128, 128], BF16, tag="mask")
            nc.sync.dma_start(t_mask[:], maskt[:])
            t_wo = {}
            for nm, ap in (("wor", wor), ("woi", woi)):
                t = qk.tile([128, 8 * 128], BF16, tag=nm, name=nm)
                nc.sync.dma_start(t[:], ap[:])
                t_wo[nm] = t
            t_ones = qk.tile([128, 1], BF16, tag="ones")
            nc.gpsimd.memset(t_ones[:], 1.0)

            Qa, Qb, Kst = [], [], []
            for hh in range(HPC):
                Qa.append(qk.tile([128, S], BF16, tag=f"Qa{hh}", name=f"Qa{hh}"))
                Qb.append(qk.tile([128, S], BF16, tag=f"Qb{hh}", name=f"Qb{hh}"))
                Kst.append(qk.tile([128, S], BF16, tag=f"Kst{hh}", name=f"Kst{hh}"))
            Vb = qk.tile([128, 16 * 256], BF16, tag="Vb")

            # ================= phase 1: projections (xT live) =============
            with tc.tile_pool(name="xp", bufs=1) as xp, \
                 tc.tile_pool(name="ws", bufs=1) as wsp:
                t_xr = xp.tile([128, 8 * S], BF16, tag="xr")
                t_xi = xp.tile([128, 8 * S], BF16, tag="xi")
                nc.sync.dma_start(t_xr[:], xtr[:])
                nc.sync.dma_start(t_xi[:], xti[:])
                t_wv = {}
                for nm, ap in (("wvr", wvr), ("wvi", wvi)):
                    t = xp.tile([128, 8 * 128], BF16, tag=nm, name=nm)
                    nc.sync.dma_start(t[:], ap[:])
                    t_wv[nm] = t

                # ---------- V projections ----------
                for ri, (wt, xs) in enumerate((("wvr", t_xr), ("wvi", t_xi))):
                    for sc in range(16):
                        pv = ps_sc.tile([128, 512], F32, tag="sc")
                        for k in range(8):
                            nc.tensor.matmul(
                                pv[:, 0:128],
                                xs[:, k * S + sc * 128: k * S + sc * 128 + 128],
                                t_wv[wt][:, k * 128:(k + 1) * 128],
                                start=(k == 0), stop=(k == 7))
                        for hh in range(HPC):
                            nc.scalar.activation(
                                Vb[:, sc * 256 + hh * 128 + ri * 64:
                                   sc * 256 + hh * 128 + ri * 64 + 64],
                                pv[:, hh * 64:(hh + 1) * 64], AF.Copy)

                # ---------- Q/K projections + rope + phase + stacks -------
                for hh in range(HPC):
                    t_bq = wsp.tile([128, 1], F32, tag="biasq")
                    nc.sync.dma_start(t_bq[:], dt_in[f"biasq{hh}"][:])
                    t_bk = wsp.tile([128, 1], F32, tag="biask")
                    nc.sync.dma_start(t_bk[:], dt_in[f"biask{hh}"][:])
                    t_ph1 = wsp.tile([128, 1], F32, tag="ph1")
                    nc.sync.dma_start(t_ph1[:], dt_in[f"ph1{hh}"][:])
                    t_ph2 = wsp.tile([128, 1], F32, tag="ph2")
                    nc.sync.dma_start(t_ph2[:], dt_in[f"ph2{hh}"][:])
                    t_w = {}
                    for w in ("wqr", "wqi", "wkr", "wki"):
                        t = wsp.tile([128, 8 * 64], BF16, tag=f"sl{w}", name=f"sl{w}")
                        nc.sync.dma_start(t[:], dt_in[f"{w}{hh}"][:])
                        t_w[w] = t

                    PRE = {}
                    for (wa, wb, bias, pre) in (
                            ("wqr", "wqi", t_bq, "Q"),
                            ("wkr", "wki", t_bk, "K")):
                        prt = wsp.tile([128, S], BF16, tag=f"PRE{pre}", name=f"PRE{pre}")
                        for blk in range(4):
                            pq = ps_sc.tile([128, 512], F32, tag="sc")
                            for k in range(8):
                                rs = slice(k * S + blk * 512,
                                           k * S + blk * 512 + 512)
                                nc.tensor.matmul(
                                    pq[0:64, :], t_w[wa][:, k * 64:(k + 1) * 64],
                                    t_xr[:, rs], start=(k == 0), stop=(k == 7),
                                    tile_position=(0, 0))
                                nc.tensor.matmul(
                                    pq[64:128, :], t_w[wb][:, k * 64:(k + 1) * 64],
                                    t_xi[:, rs], start=(k == 0), stop=(k == 7),
                                    tile_position=(0, 64))
                            nc.scalar.activation(prt[:, blk * 512:(blk + 1) * 512],
                                                 pq[:], AF.Identity,
                                                 bias=bias[:])
                        PRE[pre] = prt

                    SW = wsp.tile([128, S], BF16, tag="SW")
                    PSEL = wsp.tile([128, S], BF16, tag="PSEL")
                    for qi, pre in ((0, "Q"), (1, "K")):
                        src = PRE[pre]
                        nc.sync.dma_start(PSEL[64 * qi:64 * qi + 32, :], src[0:32, :])
                        nc.sync.dma_start(PSEL[64 * qi + 32:64 * qi + 64, :],
                                          src[64:96, :])
                        nc.sync.dma_start(SW[64 * qi:64 * qi + 16, :], src[16:32, :])
                        nc.sync.dma_start(SW[64 * qi + 16:64 * qi + 32, :],
                                          src[0:16, :])
                        nc.sync.dma_start(SW[64 * qi + 32:64 * qi + 48, :],
                                          src[80:96, :])
                        nc.sync.dma_start(SW[64 * qi + 48:64 * qi + 64, :],
                                          src[64:80, :])

                    MIX = wsp.tile([128, S], BF16, tag="MIX")
                    tt1 = wsp.tile([128, S], BF16, tag="tt1")
                    nc.vector.tensor_mul(tt1[:], PSEL[:], t_c14[:])
                    nc.vector.tensor_mul(MIX[:], SW[:], t_c24[:])
                    nc.vector.tensor_add(MIX[:], MIX[:], tt1[:])

                    nc.sync.dma_start(Qb[hh][0:32, :], MIX[0:32, :])
                    nc.sync.dma_start(Qb[hh][32:64, :], PRE["Q"][32:64, :])
                    nc.sync.dma_start(Qb[hh][64:96, :], MIX[32:64, :])
                    nc.sync.dma_start(Qb[hh][96:128, :], PRE["Q"][96:128, :])
                    nc.sync.dma_start(Qa[hh][0:32, :], MIX[32:64, :])
                    nc.sync.dma_start(Qa[hh][32:64, :], PRE["Q"][96:128, :])
                    nc.sync.dma_start(Qa[hh][64:96, :], MIX[0:32, :])
                    nc.sync.dma_start(Qa[hh][96:128, :], PRE["Q"][32:64, :])
                    nc.vector.tensor_scalar_mul(Qa[hh][0:64, :],
                                                Qa[hh][0:64, :], -1.0)

                    KR2 = wsp.tile([128, S], BF16, tag="KR2")
                    KI2 = wsp.tile([128, S], BF16, tag="KI2")
                    for half in range(2):
                        o = 64 * half
                        nc.sync.dma_start(KR2[o:o + 32, :], MIX[64:96, :])
                        nc.sync.dma_start(KR2[o + 32:o + 64, :], PRE["K"][32:64, :])
                        nc.sync.dma_start(KI2[o:o + 32, :], MIX[96:128, :])
                        nc.sync.dma_start(KI2[o + 32:o + 64, :], PRE["K"][96:128, :])
                    tki = wsp.tile([128, S], BF16, tag="tki")
                    nc.vector.tensor_scalar_mul(tki[:], KI2[:], t_ph2[:])
                    nc.vector.scalar_tensor_tensor(
                        Kst[hh][:], KR2[:], t_ph1[:], tki[:],
                        mybir.AluOpType.mult, mybir.AluOpType.add)

            # ================= phase 2: attention + outproj ===============
            with tc.tile_pool(name="mb", bufs=1) as mbp, \
                 tc.tile_pool(name="vb", bufs=3) as vbp, \
                 tc.tile_pool(name="pt", bufs=6) as ptp, \
                 tc.tile_pool(name="sm", bufs=4) as smp, \
                 tc.tile_pool(name="og", bufs=3) as ogp:
                YN = [mbp.tile([128, S], BF16, tag=f"YN{hh}", name=f"YN{hh}")
                      for hh in range(HPC)]

                for sig in range(4):
                    ntau = 4 * (sig + 1)
                    MB = [mbp.tile([128, 16 * 512], BF16, tag=f"MB{hh}", name=f"MB{hh}")
                          for hh in range(HPC)]
                    # --- scores + mag2 + sqrt (sqrt table) ---
                    for hh in range(HPC):
                        for tau in range(ntau):
                            diag = tau >= 4 * sig
                            cst = (tau - 4 * sig) * 128 if diag else 0
                            scol = slice(sig * 512 + cst, (sig + 1) * 512)
                            psa = ps_sc.tile([128, 512], F32, tag="sc")
                            psb = ps_sc.tile([128, 512], F32, tag="sc")
                            kt = Kst[hh][:, tau * 128:(tau + 1) * 128]
                            nc.tensor.matmul(psa[:, cst:512], kt, Qa[hh][:, scol],
                                             start=True, stop=True)
                            nc.tensor.matmul(psb[:, cst:512], kt, Qb[hh][:, scol],
                                             start=True, stop=True)
                            aib = vbp.tile([128, 512], BF16, tag="aib")
                            nc.vector.tensor_copy(aib[:, cst:512], psb[:, cst:512])
                            vtile = vbp.tile([128, 512], BF16, tag="vtile")
                            nc.vector._custom_dve(
                                MAG2, out=vtile[:, cst:512], in0=psa[:, cst:512],
                                in1=aib[:, cst:512], s0=magsc, s1=1e-6)
                            nc.scalar.activation(
                                MB[hh][:, tau * 512 + cst:(tau + 1) * 512],
                                vtile[:, cst:512], AF.Sqrt)

                    # --- exp (exp table) + mask + PV + sums ---
                    sums = ps_mx.tile([64, 512], F32, tag="mx")
                    yt = [ps_yt.tile([128, 512], F32, tag="yt", name="yt")
                          for _ in range(HPC)]
                    for hh in range(HPC):
                        for tau in range(ntau):
                            diag = tau >= 4 * sig
                            cst = (tau - 4 * sig) * 128 if diag else 0
                            pt = ptp.tile([128, 512], BF16, tag="pt")
                            if cst:
                                nc.gpsimd.memset(pt[:, 0:cst], 0.0)
                            nc.scalar.activation(
                                pt[:, cst:512],
                                MB[hh][:, tau * 512 + cst:(tau + 1) * 512],
                                AF.Exp, scale=alpha)
                            if diag:
                                nc.vector.tensor_mul(pt[:, cst:cst + 128],
                                                     pt[:, cst:cst + 128],
                                                     t_mask[:])
                            nc.tensor.matmul(
                                yt[hh][:],
                                Vb[:, tau * 256 + hh * 128:
                                   tau * 256 + hh * 128 + 128],
                                pt[:], start=(tau == 0), stop=(tau == ntau - 1))
                            nc.tensor.matmul(
                                sums[32 * hh:32 * hh + 1, cst:512],
                                t_ones[:], pt[:, cst:512],
                                start=(tau == 0), stop=(tau == ntau - 1),
                                tile_position=(0, 32 * hh))
                    # --- normalize ---
                    for hh in range(HPC):
                        sr = smp.tile([1, 512], F32, tag="sr")
                        nc.scalar.activation(sr[:], sums[32 * hh:32 * hh + 1, :],
                                             AF.Copy)
                        rc = smp.tile([1, 512], F32, tag="rc")
                        nc.vector.reciprocal_approx_fast(rc[:], sr[:])
                        rb = smp.tile([128, 512], F32, tag="rb")
                        nc.gpsimd.partition_broadcast(rb[:], rc[:])
                        nc.vector.tensor_mul(YN[hh][:, sig * 512:(sig + 1) * 512],
                                             yt[hh][:], rb[:])

                    # --- AllGather of this sigma's yT slice ---
                    gin = dram.tile([256, 512], BF16, tag=f"gin{sig}", name=f"gin{sig}")
                    gout = dram.tile([NCORES * 256, 512], BF16, tag=f"gout{sig}", name=f"gout{sig}")
                    sl = slice(sig * 512, (sig + 1) * 512)
                    nc.sync.dma_start(gin[0:64, :], YN[0][0:64, sl])
                    nc.sync.dma_start(gin[64:128, :], YN[1][0:64, sl])
                    nc.sync.dma_start(gin[128:192, :], YN[0][64:128, sl])
                    nc.sync.dma_start(gin[192:256, :], YN[1][64:128, sl])
                    nc.gpsimd.collective_compute(
                        "AllGather", mybir.AluOpType.bypass,
                        replica_groups=[list(range(NCORES))],
                        ins=[gin.opt()], outs=[gout.opt()])

                    # --- outproj for this sigma (128 out-cols per core) ---
                    for ri, wo in ((0, "wor"), (1, "woi")):
                        lks = []
                        for k in range(8):
                            lk = ogp.tile([128, 512], BF16, tag=f"lk{k}", name=f"lk{k}")
                            nc.sync.dma_start(
                                lk[:], gout[256 * k + 128 * ri:
                                            256 * k + 128 * ri + 128, :])
                            lks.append(lk)
                        for m in range(4):
                            po = ps_mx.tile([128, 128], F32, tag="mx")
                            for k in range(8):
                                nc.tensor.matmul(
                                    po[:], lks[k][:, m * 128:(m + 1) * 128],
                                    t_wo[wo][:, k * 128:(k + 1) * 128],
                                    start=(k == 0), stop=(k == 7))
                            ot = ogp.tile([128, 128], F32, tag="ot")
                            nc.scalar.activation(ot[:], po[:], AF.Copy)
                            dst = out_r if ri == 0 else out_i
                            nc.sync.dma_start(
                                dst[sig * 512 + m * 128:
                                    sig * 512 + (m + 1) * 128, :], ot[:])

    nc.compile()
    return nc


# --------------------------------------------------------------------------
# entry point
# --------------------------------------------------------------------------

def kernel(**inputs):
    from concourse.bass_utils import run_bass_kernel_spmd
    in_maps, bo_r, bo_i, magsc, alpha = _prep(inputs)
    key = (round(magsc, 9), round(alpha, 9))
    if key not in _CACHE:
        _CACHE[key] = _build_nc(magsc, alpha)
    nc = _CACHE[key]
    res = run_bass_kernel_spmd(nc, in_maps, core_ids=list(range(NCORES)))
    out_r = np.concatenate([np.asarray(r["out_r"], np.float32)
                            for r in res.results], axis=1) + bo_r
    out_i = np.concatenate([np.asarray(r["out_i"], np.float32)
                            for r in res.results], axis=1) + bo_i
    return out_r[None].astype(np.float32), out_i[None].astype(np.float32)


if __name__ == "__main__":
    import reference as R
    inputs = {k: np.asarray(v) for k, v in R.setup_inputs().items()}
    er, ei = [np.asarray(x) for x in R.reference(**R.setup_inputs())]
    ar, ai = kernel(**inputs)
    sc = max(np.abs(er).std(), np.abs(ei).std())
    rel = max(np.sqrt(((ar - er) ** 2).mean()) / sc,
              np.sqrt(((ai - ei) ** 2).mean()) / sc)
    print(f"Relative error: {rel:.3e}")
